# revision 15
# baseline (speedup 1.0000x reference)
"""Trainium2 Bass kernel for HTM spatial-pooler overlap + global top-k inhibition.

Problem (nn_HTMModel_19834158973432):
    overlap  = connections @ input_vector          # [4096] = [4096, 32768] @ [32768]
    boosted  = overlap * boosting_factors          # [4096]
    winners  = top_k(boosted, 82)                  # ties broken by lower index
    active   = one_hot(winners)                    # [4096] 0/1 mask
    returns (active, active * boosted)

Strategy (8 NeuronCores, SPMD):
  - connections / input_vector are exactly 0/1-valued, so the host re-encodes
    them losslessly as bit-packed uint16 (16 input positions per lane): the
    64 MiB/core f32 shard becomes a 2 MiB/core bit matrix.
  - Each core's overlap slice is a DVE SWAR popcount of (pconn & vpack):
    bitwise stages run on u32-bitcast views (exact), arithmetic stages on
    u16 (exact: DVE int arith is f32-backed, values < 2^24), and the final
    per-row accumulation runs on the ACT engine (Copy + accum_out).
  - Each core builds its local key slice
        key[c] = boosted[c] * 4096 + (4095 - c)
    (boosted is integer-valued and < 2048 -> keys are distinct exact-int
    floats; `key >= T82` reproduces top_k's lower-index tie-breaking),
    then AllGathers the 8x512 key slices (2 KB/rank).
  - Every core (redundantly) runs a branch-free 4-level 128-ary threshold
    search for the 82nd-largest key, directly on the [128, 32] key layout
    (no whole-key-set broadcast): per level one stride-0-broadcast is_ge
    pass builds a [32key x 128edge] mask per partition, 32 chained
    ones-stationary bf16 matmuls count+broadcast all 128 edge counts, and
    a tiny reduce picks the bracket.
  - boosted is reconstructed from keys as (key - negidx)/4096 (exact).
  - Each core writes the full [2, 4096] output; the host returns core 0's.
"""

import sys

if "/opt/trn_rl_repo" not in sys.path:
    sys.path.insert(0, "/opt/trn_rl_repo")

import numpy as np

C_TOT = 4096          # minicolumns
IN = 32768            # input size
CORES = 8
ROWS = C_TOT // CORES  # 512 rows per core
K_ACT = 82            # active columns per inhibition area
RB = ROWS // 128      # 4 row blocks of 128 partitions per core
G = IN // 16          # 2048 packed uint16 groups along the input axis

WIDTHS = [131072, 2048, 32, 1]  # 4-level 64-ary search over keys in [0, 2^23)
NEDGE = 64                      # edges tested per level
WSUM = 133153                   # sum of widths


def _build_nc(stage=4):
    from concourse import bacc, mybir, tile
    from concourse.ap import AP

    f32 = mybir.dt.float32
    u16 = mybir.dt.uint16
    u32 = mybir.dt.uint32
    bf16 = mybir.dt.bfloat16
    Alu = mybir.AluOpType

    nc = bacc.Bacc("TRN2", target_bir_lowering=False, debug=False,
                   enable_asserts=False, num_devices=CORES)

    pconn = nc.dram_tensor("pconn", [ROWS, G], u16, kind="ExternalInput")
    vpack = nc.dram_tensor("vpack", [G], u16, kind="ExternalInput")
    boost4 = nc.dram_tensor("boost4", [ROWS], f32, kind="ExternalInput")
    neg4 = nc.dram_tensor("neg4", [ROWS], f32, kind="ExternalInput")
    rampsf = nc.dram_tensor("rampsf", [4 * NEDGE], f32, kind="ExternalInput")
    negidx = nc.dram_tensor("negidx", [C_TOT], f32, kind="ExternalInput")
    out = nc.dram_tensor("out", [2, C_TOT], f32, kind="ExternalOutput")

    with tile.TileContext(nc) as tc:
        with (
            tc.tile_pool(name="const", bufs=1) as constp,
            tc.tile_pool(name="cpool", bufs=2) as cpool,
            tc.tile_pool(name="scrp", bufs=2) as scrp,
            tc.tile_pool(name="dramp", bufs=1, space="DRAM") as dramp,
            tc.tile_pool(name="vpsp", bufs=2, space="PSUM") as psp,
        ):
            # packed input vector broadcast to all partitions (512 KB DMA)
            vb = constp.tile([128, G], u16, name="vb")
            nc.sync.dma_start(vb[:], vpack.ap().partition_broadcast(128))
            pts = []
            for cb in range(RB):
                pt = cpool.tile([128, G], u16, name=f"pt_{cb}", tag="pt")
                nc.sync.dma_start(
                    pt[:], pconn.ap()[cb * 128:(cb + 1) * 128, :])
                pts.append(pt)
            boost4t = constp.tile([128, RB], f32, name="boost4t")
            nc.sync.dma_start(boost4t[:],
                              boost4.ap().rearrange("(c p) -> p c", p=128))
            neg4t = constp.tile([128, RB], f32, name="neg4t")
            nc.sync.dma_start(neg4t[:],
                              neg4.ap().rearrange("(c p) -> p c", p=128))
            ones_bf = constp.tile([128, 128], bf16, name="ones_bf")
            nc.vector.memset(ones_bf[:], 1.0)
            # per-level edge ramps replicated on all partitions
            rampr = constp.tile([128, 4 * NEDGE], f32, name="rampr")
            nc.sync.dma_start(rampr[:], rampsf.ap().partition_broadcast(128))
            negidx32 = constp.tile([128, 32], f32, name="negidx32")
            nc.sync.dma_start(negidx32[:],
                              negidx.ap().rearrange("(p f) -> p f", p=128))

            ova = constp.tile([128, RB], f32, name="ova")
            ovb = constp.tile([128, RB], f32, name="ovb")
            actscr = constp.tile([128, G], u16, name="actscr")

            # ---- packed popcount matvec: 4 row blocks of 128 rows ----
            for cb in range(RB):
                pt = pts[cb]
                # x = conn & v  (u32 view: bitwise ops are exact in u32)
                x = scrp.tile([128, G], u16, name=f"x_{cb}", tag="x")
                nc.vector.tensor_tensor(x[:].bitcast(u32), pt[:].bitcast(u32),
                                        vb[:].bitcast(u32), Alu.bitwise_and)
                # SWAR popcount: x1 = x - ((x >> 1) & 0x5555)
                t = scrp.tile([128, G], u16, name=f"t_{cb}", tag="t")
                nc.vector.tensor_scalar(
                    out=t[:], in0=x[:], scalar1=1, scalar2=0x5555,
                    op0=Alu.logical_shift_right, op1=Alu.bitwise_and)
                x1 = scrp.tile([128, G], u16, name=f"x1_{cb}", tag="x1")
                nc.vector.tensor_tensor(x1[:], x[:], t[:], Alu.subtract)
                # x2 = (x1 & 0x3333) + ((x1 >> 2) & 0x3333)
                t2 = scrp.tile([128, G], u16, name=f"t2_{cb}", tag="t2")
                nc.vector.tensor_scalar(
                    out=t2[:], in0=x1[:], scalar1=2, scalar2=0x3333,
                    op0=Alu.logical_shift_right, op1=Alu.bitwise_and)
                x1m = scrp.tile([128, G], u16, name=f"x1m_{cb}", tag="x1m")
                nc.vector.tensor_scalar(
                    out=x1m[:], in0=x1[:], scalar1=0x3333, scalar2=None,
                    op0=Alu.bitwise_and)
                x2 = scrp.tile([128, G], u16, name=f"x2_{cb}", tag="x2")
                nc.vector.tensor_tensor(x2[:], x1m[:], t2[:], Alu.add)
                # x3 = x2 + (x2 >> 4): nibble0 = bits0-7 count, nib2 = bits8-15
                t3 = scrp.tile([128, G], u16, name=f"t3_{cb}", tag="t3")
                nc.vector.tensor_scalar(
                    out=t3[:], in0=x2[:], scalar1=4, scalar2=None,
                    op0=Alu.logical_shift_right)
                x3 = scrp.tile([128, G], u16, name=f"x3_{cb}", tag="x3")
                nc.vector.tensor_tensor(x3[:], x2[:], t3[:], Alu.add)
                # extract both byte-counts; accumulate each on the ACT engine
                m0 = scrp.tile([128, G], u16, name=f"m0_{cb}", tag="m0")
                nc.vector.tensor_scalar(
                    out=m0[:], in0=x3[:], scalar1=0x0F, scalar2=None,
                    op0=Alu.bitwise_and)
                m1 = scrp.tile([128, G], u16, name=f"m1_{cb}", tag="m1")
                nc.vector.tensor_scalar(
                    out=m1[:], in0=x3[:], scalar1=8, scalar2=0x0F,
                    op0=Alu.logical_shift_right, op1=Alu.bitwise_and)
                nc.scalar.activation(actscr[:], m0[:],
                                     mybir.ActivationFunctionType.Copy,
                                     accum_out=ova[:, cb:cb + 1])
                nc.scalar.activation(actscr[:], m1[:],
                                     mybir.ActivationFunctionType.Copy,
                                     accum_out=ovb[:, cb:cb + 1])

            ov4 = constp.tile([128, RB], f32, name="ov4")
            nc.vector.tensor_tensor(ov4[:], ova[:], ovb[:], Alu.add)

            # ---- local keys: key = overlap*boost*4096 + (4095 - c) ----
            key4 = constp.tile([128, RB], f32, name="key4")
            nc.vector.tensor_tensor(key4[:], ov4[:], boost4t[:], Alu.mult)
            nc.vector.tensor_scalar(
                out=key4[:], in0=key4[:], scalar1=4096.0, scalar2=None,
                op0=Alu.mult)
            nc.vector.tensor_tensor(key4[:], key4[:], neg4t[:], Alu.add)

            if stage <= 1:
                nc.sync.dma_start(
                    out.ap()[0][0:ROWS].rearrange("(c p) -> p c", p=128),
                    key4[:])
                nc.sync.dma_start(
                    out.ap()[1][0:ROWS].rearrange("(c p) -> p c", p=128),
                    ov4[:])
            if stage >= 2:
                cc_in = dramp.tile([ROWS], f32, name="cc_in")
                cc_out = dramp.tile([C_TOT], f32, name="cc_out",
                                    addr_space="Shared")
                # local c = cb*128 + p  ->  dram[(c b) ...] viewed [p, cb]
                nc.sync.dma_start(cc_in.rearrange("(c p) -> p c", p=128),
                                  key4[:])
                nc.gpsimd.collective_compute(
                    "AllGather", Alu.bypass,
                    replica_groups=[list(range(CORES))],
                    ins=[cc_in.opt()],
                    outs=[cc_out.opt()],
                )
                # gathered keys on the [128, 32] layout (c = p*32 + f)
                key32 = constp.tile([128, 32], f32, name="key32")
                nc.sync.dma_start(key32[:],
                                  cc_out.rearrange("(p f) -> p f", p=128))
                # boosted = (key - (4095-c)) / 4096, exact
                boosted32 = constp.tile([128, 32], f32, name="boosted32")
                nc.vector.tensor_tensor(boosted32[:], key32[:], negidx32[:],
                                        Alu.subtract)
                nc.vector.tensor_scalar(
                    out=boosted32[:], in0=boosted32[:],
                    scalar1=1.0 / 4096.0, scalar2=None, op0=Alu.mult)

            if stage == 2:
                nc.sync.dma_start(
                    out.ap()[0].rearrange("(p f) -> p f", p=128), key32[:])
                nc.sync.dma_start(
                    out.ap()[1].rearrange("(p f) -> p f", p=128),
                    boosted32[:])

            if stage >= 3:
                # ---- 4-level 64-ary threshold search on the [128, 32]
                # layout: mask[p, kf, e] = (key32[p, kf] >= edges[e]) ----
                # A_l = sum_{j<=l} w_j*cnt_j ; T = A_3 - WSUM
                key_bc = AP(key32[:].tensor, key32[:].offset,
                            [key32[:].ap[0], [1, 32], [0, NEDGE]])
                acur = None
                for li, w in enumerate(WIDTHS):
                    if li == 0:
                        edges = rampr[:, 0:NEDGE]
                    else:
                        e2 = constp.tile([128, NEDGE], f32, name=f"edges{li}")
                        nc.vector.tensor_scalar(
                            out=e2[:],
                            in0=rampr[:, li * NEDGE:(li + 1) * NEDGE],
                            scalar1=acur[:], scalar2=None, op0=Alu.add)
                        edges = e2[:]
                    edges_bc = AP(edges.tensor, edges.offset,
                                  [edges.ap[0], [0, 32], [1, NEDGE]])
                    mask = scrp.tile([128, 32, NEDGE], bf16, name=f"mask{li}",
                                     tag="mask", bufs=1)
                    nc.vector.tensor_tensor(mask[:], key_bc, edges_bc,
                                            Alu.is_ge)
                    # count: psum[p, (kfg, e)] = partial sums over 8-kf groups
                    cnt_ps = psp.tile([128, 8, NEDGE], f32, name=f"cnt{li}",
                                      tag="vps")
                    for g in range(4):
                        nc.tensor.matmul(
                            cnt_ps[:].opt(), lhsT=ones_bf[:],
                            rhs=mask[:, 8 * g:8 * (g + 1), :].opt(),
                            start=(g == 0), stop=(g == 3))
                    # fold the 8 kf-groups: view [p, e, kfg], reduce last dim
                    cnt_t = AP(cnt_ps[:].tensor, cnt_ps[:].offset,
                               [cnt_ps[:].ap[0], [1, NEDGE], [NEDGE, 8]])
                    tot = constp.tile([128, NEDGE], f32, name=f"tot{li}")
                    nc.vector.reduce_sum(tot[:], cnt_t,
                                         axis=mybir.AxisListType.X)
                    # cnt_l = #edges with count >= K (a prefix) -> [128, 1]
                    selscr = constp.tile([128, NEDGE], f32, name=f"sel{li}")
                    cnt = constp.tile([128, 1], f32, name=f"cntv{li}")
                    nc.vector.tensor_scalar(
                        out=selscr[:], in0=tot[:], scalar1=float(K_ACT),
                        scalar2=None, op0=Alu.is_ge, op1=Alu.add,
                        accum_out=cnt[:])
                    anew = constp.tile([128, 1], f32, name=f"a{li}")
                    if li == 0:
                        nc.vector.tensor_scalar(
                            out=anew[:], in0=cnt[:], scalar1=float(w),
                            scalar2=None, op0=Alu.mult)
                    else:
                        nc.vector.tensor_scalar(
                            out=anew[:], in0=cnt[:], scalar1=float(w),
                            scalar2=acur[:], op0=Alu.mult, op1=Alu.add)
                    acur = anew

                tthr = constp.tile([128, 1], f32, name="tthr")
                nc.vector.tensor_scalar(
                    out=tthr[:], in0=acur[:], scalar1=float(-WSUM),
                    scalar2=None, op0=Alu.add)

                # ---- apply threshold, write outputs (contiguous) ----
                active32 = constp.tile([128, 32], f32, name="active32")
                nc.vector.tensor_scalar(
                    out=active32[:], in0=key32[:], scalar1=tthr[:],
                    scalar2=None, op0=Alu.is_ge,
                )
                masked32 = constp.tile([128, 32], f32, name="masked32")
                nc.vector.tensor_tensor(masked32[:], active32[:],
                                        boosted32[:], Alu.mult)
                nc.sync.dma_start(
                    out.ap()[0].rearrange("(p f) -> p f", p=128), active32[:])
                nc.sync.dma_start(
                    out.ap()[1].rearrange("(p f) -> p f", p=128), masked32[:])

    nc.compile()
    return nc


def _pack_bits_u16(a):
    """[..., N] 0/1 f32 -> [..., N/16] uint16, bit t of group g = a[16g+t]."""
    b = np.packbits(a.astype(np.uint8), axis=-1, bitorder="little")
    return b.view("<u2").reshape(*a.shape[:-1], a.shape[-1] // 16)


def _make_in_maps(input_vector, connections, boosting_factors):
    v = np.ascontiguousarray(np.asarray(input_vector, dtype=np.float32))
    c = np.asarray(connections, dtype=np.float32)
    b = np.ascontiguousarray(np.asarray(boosting_factors, dtype=np.float32))
    vp = np.ascontiguousarray(_pack_bits_u16(v))
    neg = (float(C_TOT - 1) - np.arange(C_TOT, dtype=np.float32))
    # per-level edge ramps (along free axis) with cumulative -w folded in
    rampsf = np.zeros((4, NEDGE), dtype=np.float32)
    csum = 0.0
    for li, w in enumerate(WIDTHS):
        rampsf[li] = np.arange(NEDGE, dtype=np.float32) * w - csum
        csum += w
    maps = []
    for r in range(CORES):
        sh = np.ascontiguousarray(
            _pack_bits_u16(c[r * ROWS:(r + 1) * ROWS]))
        maps.append({
            "pconn": sh,
            "vpack": vp,
            "boost4": np.ascontiguousarray(b[r * ROWS:(r + 1) * ROWS]),
            "neg4": np.ascontiguousarray(neg[r * ROWS:(r + 1) * ROWS]),
            "rampsf": np.ascontiguousarray(rampsf.reshape(-1)),
            "negidx": neg,
        })
    return maps


def _run(input_vector, connections, boosting_factors, trace=False, stage=4):
    from concourse import bass_utils

    nc = _build_nc(stage)
    in_maps = _make_in_maps(input_vector, connections, boosting_factors)
    res = bass_utils.run_bass_kernel_spmd(
        nc, in_maps, core_ids=list(range(CORES)), trace=trace,
    )
    out = res.results[0]["out"]
    return (np.ascontiguousarray(out[0]), np.ascontiguousarray(out[1])), res


def kernel(input_vector, connections, boosting_factors):
    (active, masked), _ = _run(input_vector, connections, boosting_factors)
    return active, masked


# revision 16
# speedup vs baseline: 1.1043x; 1.1043x over previous
"""Trainium2 Bass kernel for HTM spatial-pooler overlap + global top-k inhibition.

Problem (nn_HTMModel_19834158973432):
    overlap  = connections @ input_vector          # [4096] = [4096, 32768] @ [32768]
    boosted  = overlap * boosting_factors          # [4096]
    winners  = top_k(boosted, 82)                  # ties broken by lower index
    active   = one_hot(winners)                    # [4096] 0/1 mask
    returns (active, active * boosted)

Strategy (8 NeuronCores, SPMD):
  - connections / input_vector are exactly 0/1-valued, so the host re-encodes
    them losslessly as bit-packed uint16 (16 input positions per lane): the
    64 MiB/core f32 shard becomes a 2 MiB/core bit matrix.
  - Each core's overlap slice is a DVE SWAR popcount of (pconn & vpack):
    bitwise stages run on u32-bitcast views (exact), arithmetic stages on
    u16 (exact: DVE int arith is f32-backed, values < 2^24), and the final
    per-row accumulation runs on the ACT engine (Copy + accum_out).
  - Each core builds its local key slice
        key[c] = boosted[c] * 4096 + (4095 - c)
    (boosted is integer-valued and < 2048 -> keys are distinct exact-int
    floats; `key >= T82` reproduces top_k's lower-index tie-breaking),
    then AllGathers the 8x512 key slices (2 KB/rank).
  - Every core (redundantly) runs a branch-free 4-level 64-ary threshold
    search for the 82nd-largest key, directly on the [128, 32] key layout
    (no whole-key-set broadcast): per level one stride-0-broadcast is_ge
    pass builds a [32key x 64edge] bf16 mask per partition, 4 chained
    ones-stationary matmuls count all 64 edges across partitions (result
    broadcast to every partition for free), and a tiny reduce+compare
    picks the bracket.
  - boosted is reconstructed from keys as (key - negidx)/4096 (exact).
  - Each core writes the full [2, 4096] output; the host returns core 0's.
"""

import sys

if "/opt/trn_rl_repo" not in sys.path:
    sys.path.insert(0, "/opt/trn_rl_repo")

import numpy as np

C_TOT = 4096          # minicolumns
IN = 32768            # input size
CORES = 8
ROWS = C_TOT // CORES  # 512 rows per core
K_ACT = 82            # active columns per inhibition area
RB = ROWS // 128      # 4 row blocks of 128 partitions per core
G = IN // 16          # 2048 packed uint16 groups along the input axis

WIDTHS = [131072, 2048, 32, 1]  # 4-level 64-ary search over keys in [0, 2^23)
NEDGE = 64                      # edges tested per level
WSUM = 133153                   # sum of widths


def _build_nc(stage=4):
    from concourse import bacc, mybir, tile
    from concourse.ap import AP

    f32 = mybir.dt.float32
    u16 = mybir.dt.uint16
    u32 = mybir.dt.uint32
    bf16 = mybir.dt.bfloat16
    Alu = mybir.AluOpType

    nc = bacc.Bacc("TRN2", target_bir_lowering=False, debug=False,
                   enable_asserts=False, num_devices=CORES)

    pconn = nc.dram_tensor("pconn", [ROWS, G], u16, kind="ExternalInput")
    vpack = nc.dram_tensor("vpack", [G], u16, kind="ExternalInput")
    boost4 = nc.dram_tensor("boost4", [ROWS], f32, kind="ExternalInput")
    neg4 = nc.dram_tensor("neg4", [ROWS], f32, kind="ExternalInput")
    rampsf = nc.dram_tensor("rampsf", [4 * NEDGE], f32, kind="ExternalInput")
    negidx = nc.dram_tensor("negidx", [C_TOT], f32, kind="ExternalInput")
    out = nc.dram_tensor("out", [2, C_TOT], f32, kind="ExternalOutput")

    with tile.TileContext(nc) as tc:
        with (
            tc.tile_pool(name="const", bufs=1) as constp,
            tc.tile_pool(name="cpool", bufs=2) as cpool,
            tc.tile_pool(name="scrp", bufs=2) as scrp,
            tc.tile_pool(name="dramp", bufs=1, space="DRAM") as dramp,
            tc.tile_pool(name="vpsp", bufs=2, space="PSUM") as psp,
        ):
            # packed input vector broadcast to all partitions (512 KB DMA)
            vb = constp.tile([128, G], u16, name="vb")
            nc.sync.dma_start(vb[:], vpack.ap().partition_broadcast(128))
            pts = []
            for cb in range(RB):
                pt = cpool.tile([128, G], u16, name=f"pt_{cb}", tag="pt")
                nc.sync.dma_start(
                    pt[:], pconn.ap()[cb * 128:(cb + 1) * 128, :])
                pts.append(pt)
            boost4t = constp.tile([128, RB], f32, name="boost4t")
            nc.sync.dma_start(boost4t[:],
                              boost4.ap().rearrange("(c p) -> p c", p=128))
            neg4t = constp.tile([128, RB], f32, name="neg4t")
            nc.sync.dma_start(neg4t[:],
                              neg4.ap().rearrange("(c p) -> p c", p=128))
            ones_bf = constp.tile([128, 128], bf16, name="ones_bf")
            nc.vector.memset(ones_bf[:], 1.0)
            # per-level edge ramps replicated on all partitions
            rampr = constp.tile([128, 4 * NEDGE], f32, name="rampr")
            nc.sync.dma_start(rampr[:], rampsf.ap().partition_broadcast(128))
            negidx32 = constp.tile([128, 32], f32, name="negidx32")
            nc.sync.dma_start(negidx32[:],
                              negidx.ap().rearrange("(p f) -> p f", p=128))

            ova = constp.tile([128, RB], f32, name="ova")
            ovb = constp.tile([128, RB], f32, name="ovb")
            actscr = constp.tile([128, G], u16, name="actscr")

            # ---- packed popcount matvec: 4 row blocks of 128 rows ----
            for cb in range(RB):
                pt = pts[cb]
                # x = conn & v  (u32 view: bitwise ops are exact in u32)
                x = scrp.tile([128, G], u16, name=f"x_{cb}", tag="x")
                nc.vector.tensor_tensor(x[:].bitcast(u32), pt[:].bitcast(u32),
                                        vb[:].bitcast(u32), Alu.bitwise_and)
                # SWAR popcount: x1 = x - ((x >> 1) & 0x5555)
                t = scrp.tile([128, G], u16, name=f"t_{cb}", tag="t")
                nc.vector.tensor_scalar(
                    out=t[:], in0=x[:], scalar1=1, scalar2=0x5555,
                    op0=Alu.logical_shift_right, op1=Alu.bitwise_and)
                x1 = scrp.tile([128, G], u16, name=f"x1_{cb}", tag="x1")
                nc.vector.tensor_tensor(x1[:], x[:], t[:], Alu.subtract)
                # x2 = (x1 & 0x3333) + ((x1 >> 2) & 0x3333)
                t2 = scrp.tile([128, G], u16, name=f"t2_{cb}", tag="t2")
                nc.vector.tensor_scalar(
                    out=t2[:], in0=x1[:], scalar1=2, scalar2=0x3333,
                    op0=Alu.logical_shift_right, op1=Alu.bitwise_and)
                x1m = scrp.tile([128, G], u16, name=f"x1m_{cb}", tag="x1m")
                nc.vector.tensor_scalar(
                    out=x1m[:], in0=x1[:], scalar1=0x3333, scalar2=None,
                    op0=Alu.bitwise_and)
                x2 = scrp.tile([128, G], u16, name=f"x2_{cb}", tag="x2")
                nc.vector.tensor_tensor(x2[:], x1m[:], t2[:], Alu.add)
                # x3 = x2 + (x2 >> 4): nibble0 = bits0-7 count, nib2 = bits8-15
                t3 = scrp.tile([128, G], u16, name=f"t3_{cb}", tag="t3")
                nc.vector.tensor_scalar(
                    out=t3[:], in0=x2[:], scalar1=4, scalar2=None,
                    op0=Alu.logical_shift_right)
                x3 = scrp.tile([128, G], u16, name=f"x3_{cb}", tag="x3")
                nc.vector.tensor_tensor(x3[:], x2[:], t3[:], Alu.add)
                # extract both byte-counts; accumulate each on the ACT engine
                m0 = scrp.tile([128, G], u16, name=f"m0_{cb}", tag="m0")
                nc.vector.tensor_scalar(
                    out=m0[:], in0=x3[:], scalar1=0x0F, scalar2=None,
                    op0=Alu.bitwise_and)
                m1 = scrp.tile([128, G], u16, name=f"m1_{cb}", tag="m1")
                nc.vector.tensor_scalar(
                    out=m1[:], in0=x3[:], scalar1=8, scalar2=0x0F,
                    op0=Alu.logical_shift_right, op1=Alu.bitwise_and)
                nc.scalar.activation(actscr[:], m0[:],
                                     mybir.ActivationFunctionType.Copy,
                                     accum_out=ova[:, cb:cb + 1])
                nc.scalar.activation(actscr[:], m1[:],
                                     mybir.ActivationFunctionType.Copy,
                                     accum_out=ovb[:, cb:cb + 1])

            ov4 = constp.tile([128, RB], f32, name="ov4")
            nc.vector.tensor_tensor(ov4[:], ova[:], ovb[:], Alu.add)

            # ---- local keys: key = overlap*boost*4096 + (4095 - c) ----
            key4 = constp.tile([128, RB], f32, name="key4")
            nc.vector.tensor_tensor(key4[:], ov4[:], boost4t[:], Alu.mult)
            nc.vector.tensor_scalar(
                out=key4[:], in0=key4[:], scalar1=4096.0, scalar2=None,
                op0=Alu.mult)
            nc.vector.tensor_tensor(key4[:], key4[:], neg4t[:], Alu.add)

            if stage <= 1:
                nc.sync.dma_start(
                    out.ap()[0][0:ROWS].rearrange("(c p) -> p c", p=128),
                    key4[:])
                nc.sync.dma_start(
                    out.ap()[1][0:ROWS].rearrange("(c p) -> p c", p=128),
                    ov4[:])
            if stage >= 2:
                cc_in = dramp.tile([ROWS], f32, name="cc_in")
                cc_out = dramp.tile([C_TOT], f32, name="cc_out",
                                    addr_space="Shared")
                # local c = cb*128 + p  ->  dram[(c b) ...] viewed [p, cb]
                nc.sync.dma_start(cc_in.rearrange("(c p) -> p c", p=128),
                                  key4[:])
                nc.gpsimd.collective_compute(
                    "AllGather", Alu.bypass,
                    replica_groups=[list(range(CORES))],
                    ins=[cc_in.opt()],
                    outs=[cc_out.opt()],
                )
                # gathered keys on the [128, 32] layout (c = p*32 + f)
                key32 = constp.tile([128, 32], f32, name="key32")
                nc.sync.dma_start(key32[:],
                                  cc_out.rearrange("(p f) -> p f", p=128))
                # boosted = (key - (4095-c)) / 4096, exact
                boosted32 = constp.tile([128, 32], f32, name="boosted32")
                nc.vector.tensor_tensor(boosted32[:], key32[:], negidx32[:],
                                        Alu.subtract)
                nc.vector.tensor_scalar(
                    out=boosted32[:], in0=boosted32[:],
                    scalar1=1.0 / 4096.0, scalar2=None, op0=Alu.mult)

            if stage == 2:
                nc.sync.dma_start(
                    out.ap()[0].rearrange("(p f) -> p f", p=128), key32[:])
                nc.sync.dma_start(
                    out.ap()[1].rearrange("(p f) -> p f", p=128),
                    boosted32[:])

            if stage >= 3:
                # ---- 4-level 64-ary threshold search on the [128, 32]
                # layout: mask[p, kf, e] = (key32[p, kf] >= edges[e]) ----
                # A_l = sum_{j<=l} w_j*cnt_j ; T = A_3 - WSUM
                key_bc = AP(key32[:].tensor, key32[:].offset,
                            [key32[:].ap[0], [1, 32], [0, NEDGE]])
                acur = None
                for li, w in enumerate(WIDTHS):
                    if li == 0:
                        edges = rampr[:, 0:NEDGE]
                    else:
                        e2 = constp.tile([128, NEDGE], f32, name=f"edges{li}")
                        nc.vector.tensor_scalar(
                            out=e2[:],
                            in0=rampr[:, li * NEDGE:(li + 1) * NEDGE],
                            scalar1=acur[:], scalar2=None, op0=Alu.add)
                        edges = e2[:]
                    edges_bc = AP(edges.tensor, edges.offset,
                                  [edges.ap[0], [0, 32], [1, NEDGE]])
                    mask = scrp.tile([128, 32, NEDGE], bf16, name=f"mask{li}",
                                     tag="mask", bufs=1)
                    nc.vector.tensor_tensor(mask[:], key_bc, edges_bc,
                                            Alu.is_ge)
                    # count: psum[p, (kfg, e)] = partial sums over 8-kf groups
                    cnt_ps = psp.tile([128, 8, NEDGE], f32, name=f"cnt{li}",
                                      tag="vps")
                    for g in range(4):
                        nc.tensor.matmul(
                            cnt_ps[:].opt(), lhsT=ones_bf[:],
                            rhs=mask[:, 8 * g:8 * (g + 1), :].opt(),
                            start=(g == 0), stop=(g == 3))
                    # fold the 8 kf-groups: view [p, e, kfg], reduce last dim
                    cnt_t = AP(cnt_ps[:].tensor, cnt_ps[:].offset,
                               [cnt_ps[:].ap[0], [1, NEDGE], [NEDGE, 8]])
                    tot = constp.tile([128, NEDGE], f32, name=f"tot{li}")
                    nc.vector.reduce_sum(tot[:], cnt_t,
                                         axis=mybir.AxisListType.X)
                    # cnt_l = #edges with count >= K (a prefix) -> [128, 1]
                    selscr = constp.tile([128, NEDGE], f32, name=f"sel{li}")
                    cnt = constp.tile([128, 1], f32, name=f"cntv{li}")
                    nc.vector.tensor_scalar(
                        out=selscr[:], in0=tot[:], scalar1=float(K_ACT),
                        scalar2=None, op0=Alu.is_ge, op1=Alu.add,
                        accum_out=cnt[:])
                    anew = constp.tile([128, 1], f32, name=f"a{li}")
                    if li == 0:
                        nc.vector.tensor_scalar(
                            out=anew[:], in0=cnt[:], scalar1=float(w),
                            scalar2=None, op0=Alu.mult)
                    else:
                        nc.vector.tensor_scalar(
                            out=anew[:], in0=cnt[:], scalar1=float(w),
                            scalar2=acur[:], op0=Alu.mult, op1=Alu.add)
                    acur = anew

                tthr = constp.tile([128, 1], f32, name="tthr")
                nc.vector.tensor_scalar(
                    out=tthr[:], in0=acur[:], scalar1=float(-WSUM),
                    scalar2=None, op0=Alu.add)

                # ---- apply threshold, write outputs (contiguous) ----
                active32 = constp.tile([128, 32], f32, name="active32")
                nc.vector.tensor_scalar(
                    out=active32[:], in0=key32[:], scalar1=tthr[:],
                    scalar2=None, op0=Alu.is_ge,
                )
                masked32 = constp.tile([128, 32], f32, name="masked32")
                nc.vector.tensor_tensor(masked32[:], active32[:],
                                        boosted32[:], Alu.mult)
                nc.sync.dma_start(
                    out.ap()[0].rearrange("(p f) -> p f", p=128), active32[:])
                nc.sync.dma_start(
                    out.ap()[1].rearrange("(p f) -> p f", p=128), masked32[:])

    nc.compile()
    return nc


def _pack_bits_u16(a):
    """[..., N] 0/1 f32 -> [..., N/16] uint16, bit t of group g = a[16g+t]."""
    b = np.packbits(a.astype(np.uint8), axis=-1, bitorder="little")
    return b.view("<u2").reshape(*a.shape[:-1], a.shape[-1] // 16)


def _make_in_maps(input_vector, connections, boosting_factors):
    v = np.ascontiguousarray(np.asarray(input_vector, dtype=np.float32))
    c = np.asarray(connections, dtype=np.float32)
    b = np.ascontiguousarray(np.asarray(boosting_factors, dtype=np.float32))
    vp = np.ascontiguousarray(_pack_bits_u16(v))
    neg = (float(C_TOT - 1) - np.arange(C_TOT, dtype=np.float32))
    # per-level edge ramps (along free axis) with cumulative -w folded in
    rampsf = np.zeros((4, NEDGE), dtype=np.float32)
    csum = 0.0
    for li, w in enumerate(WIDTHS):
        rampsf[li] = np.arange(NEDGE, dtype=np.float32) * w - csum
        csum += w
    maps = []
    for r in range(CORES):
        sh = np.ascontiguousarray(
            _pack_bits_u16(c[r * ROWS:(r + 1) * ROWS]))
        maps.append({
            "pconn": sh,
            "vpack": vp,
            "boost4": np.ascontiguousarray(b[r * ROWS:(r + 1) * ROWS]),
            "neg4": np.ascontiguousarray(neg[r * ROWS:(r + 1) * ROWS]),
            "rampsf": np.ascontiguousarray(rampsf.reshape(-1)),
            "negidx": neg,
        })
    return maps


def _run(input_vector, connections, boosting_factors, trace=False, stage=4):
    from concourse import bass_utils

    nc = _build_nc(stage)
    in_maps = _make_in_maps(input_vector, connections, boosting_factors)
    res = bass_utils.run_bass_kernel_spmd(
        nc, in_maps, core_ids=list(range(CORES)), trace=trace,
    )
    out = res.results[0]["out"]
    return (np.ascontiguousarray(out[0]), np.ascontiguousarray(out[1])), res


def kernel(input_vector, connections, boosting_factors):
    (active, masked), _ = _run(input_vector, connections, boosting_factors)
    return active, masked
